# revision 49
# baseline (speedup 1.0000x reference)
"""Trainium2 Bass kernel for varlen (ragged) BERT self-attention.

Strategy: tensor-parallel over heads. 16 heads across 8 NeuronCores ->
2 heads per core. Every core runs an IDENTICAL program (SPMD) on:
  - xt:   full hidden_states (sequences host-permuted to DESCENDING
          length), pre-transposed+cast to bf16, tiled [NT, 128, 8, 512]
  - wt:   this core's slice of Wqkv as matmul-lhsT blocks, mc-major
          [3, 128, 8, 128] bf16 (per-stripe DMA -> first matmul sooner)
  - bias: this core's bias slice (128, 3) f32
Output per core: (128, nnz) bf16 = the 2 owned heads' output rows,
TRANSPOSED, in permuted token order. Host transposes back, concatenates,
and un-permutes.

On-chip per core:
  1. QKV projection: Y^T[384, nnz] = Wc @ X^T, K=1024 in 8 chunks,
     bias added during PSUM->SBUF eviction (DVE tensor_scalar add),
     cast bf16 -> qT/kT/vT resident in SBUF as [128(=2hx64), nnz].
  2. Attention per sequence (descending length; NO packing, NO masks:
     only diagonal blocks are computed), one "beat" per (q-chunk of
     512, k-chunk jc of 128): the TWO heads' score matmuls (K=64) are
     issued back-to-back into opposite PE row groups (partitions 0-63
     vs 64-127 via auto tile_position) so they execute CONCURRENTLY;
     one shared exp on ACT over [nj, 2, nq] (scale 1/8 folded in);
     two out matmuls (va = [ones | pad | v] so the softmax denominator
     lands on PSUM partition 0 for free). Normalize per (q-chunk,
     head): DVE reciprocal of the denominator row, GpSimd
     partition_broadcast, DVE multiply (cast bf16), row-contiguous DMA.
  3. v_aug refresh (PE transpose of vT block + DVE copy) is PIPELINED
     INTO THE FEEDER STREAM one unit ahead of attention (va slots are
     parity double-buffered per unit), so unit boundaries never starve
     the PE -- PE starvation is what used to trip the HAM clock gate
     (k=4/8 half-clock windows). Keepalive matmuls still hold the gate
     open once the feeder runs dry in the tail.

Scheduling: the QKV chunk stream (PE-only work) is paced into the
beat stream (groups-remaining / beats-remaining), with sequences
DESCENDING so the deepest attention overlaps the longest feeder
stretch and the tail units are trivial.
"""

import functools
import sys

import numpy as np

for _p in ("/opt/trn_rl_repo",):
    if _p not in sys.path:
        sys.path.append(_p)

import ml_dtypes  # noqa: E402

N_HEADS = 16
HEAD_DIM = 64
DIM = 1024
N_CORES = 8
HEADS_PER_CORE = N_HEADS // N_CORES  # 2


@functools.lru_cache(maxsize=4)
def _build(nnz, lengths):
    """Build + compile the SPMD Bass program.

    lengths: per-sequence token counts, already in DESCENDING order
    (the host permutes sequences before tiling xt).
    """
    from collections import deque

    import concourse.mybir as mybir
    import concourse.tile as tile
    from concourse import bacc
    from concourse.masks import make_identity

    f32 = mybir.dt.float32
    bf16 = mybir.dt.bfloat16
    Exp = mybir.ActivationFunctionType.Exp
    Mult = mybir.AluOpType.mult

    KC = DIM // 128  # 8 contraction chunks
    D = HEAD_DIM
    n_tok_chunks = (nnz + 511) // 512

    units = []
    off = 0
    for L in lengths:
        if L:
            units.append((off, L))
            off += L
    assert off == nnz

    nc = bacc.Bacc("TRN2", target_bir_lowering=False, debug=False)
    xt = nc.declare_dram_parameter(
        "xt", [n_tok_chunks, 128, KC, 512], bf16, isOutput=False
    )
    wt = nc.declare_dram_parameter("wt", [3, 128, KC, 128], bf16, isOutput=False)
    bias = nc.declare_dram_parameter("bias", [128, 3], f32, isOutput=False)
    out = nc.declare_dram_parameter("out", [128, nnz], bf16, isOutput=True)

    with tile.TileContext(nc) as tc:
        with (
            tc.tile_pool(name="res", bufs=1) as res,
            tc.tile_pool(name="xp", bufs=8) as xp,
            tc.tile_pool(name="esp", bufs=6) as esp,
            tc.tile_pool(name="rsp", bufs=3) as rsp,
            tc.tile_pool(name="rbp", bufs=3) as rbp,
            tc.tile_pool(name="obp", bufs=6) as obp,
            tc.tile_pool(name="ps", bufs=1, space="PSUM") as ps,
        ):
            # startup DMAs interleaved by first-use order: the sync queue
            # serves wt stripe 0, then chunk0's second half, then stripe 1,
            # chunk1's second half, stripe 2, bias; the scalar queue serves
            # the chunks' first halves in parallel.
            wt_sb = res.tile([128, 3, KC, 128], bf16)
            bias_sb = res.tile([128, 3], f32)
            prefetched = {}
            for ti in (0, 1):
                prefetched[ti] = xp.tile([128, KC, 512], bf16, tag="xt", name="xt_t")
                nc.scalar.dma_start(prefetched[ti][:, 0:4, :], xt[ti, :, 0:4, :])
            nc.sync.dma_start(wt_sb[:, 0, :, :], wt[0, :, :, :])
            nc.sync.dma_start(prefetched[0][:, 4:8, :], xt[0, :, 4:8, :])
            nc.sync.dma_start(bias_sb[:], bias[:, :])
            nc.sync.dma_start(wt_sb[:, 1, :, :], wt[1, :, :, :])
            nc.sync.dma_start(wt_sb[:, 2, :, :], wt[2, :, :, :])
            nc.sync.dma_start(prefetched[1][:, 4:8, :], xt[1, :, 4:8, :])

            ident_bf = res.tile([128, 128], bf16)
            make_identity(nc, ident_bf[:])

            # PE warmup: dependency-free matmuls at t0 keep the tensor
            # engine busy (and its clock ramping) while the first DMAs are
            # still in flight
            warm = res.tile([128, 512], bf16)
            nc.vector.memset(warm[:, :], 0.0)
            for _ in range(20):
                wp = ps.tile([128, 512], f32, tag="mm", bufs=2, name="wp")
                nc.tensor.matmul(
                    wp[:, :], warm[:, 0:128], warm[:, :], start=True, stop=True
                )

            qT = res.tile([128, nnz], bf16)
            kT = res.tile([128, nnz], bf16)
            vT = res.tile([128, nnz], bf16)
            qkvT = (qT, kT, vT)

            # va slots, parity double-buffered per unit so the refresh for
            # unit u+1 can run during unit u with no WAR stall:
            # [ktok(128), ones(1)+pad(63)+v(64)]. Ones col 0 => softmax
            # denominator lands on PSUM partition 0; v at cols 64..127 so
            # the normalize multiply reads PSUM partitions 64..127.
            max_nk = max((L + 127) // 128 for _, L in units)
            va = {}
            for par in range(2):
                for jc in range(max_nk):
                    # both heads share one tile: [ktok, head, ones|pad|v]
                    t = res.tile([128, 2, 128], bf16, name=f"va{par}{jc}")
                    nc.vector.memset(t[:, :, :], 0.0)
                    nc.vector.memset(t[:, :, 0:1], 1.0)
                    va[(par, jc)] = t

            # --- QKV feeder: one (ti, mc) matmul group per step ---
            state = {"ti_done": 0}

            def _qkv_groups():
                for ti in range(n_tok_chunks):
                    t0 = ti * 512
                    nt = min(512, nnz - t0)
                    if ti in prefetched:
                        xt_tile = prefetched[ti]
                    else:
                        xt_tile = xp.tile(
                            [128, KC, 512], bf16, tag="xt", name="xt_t"
                        )
                        nc.sync.dma_start(xt_tile[:, :, :], xt[ti, :, :, :])
                    for mc in range(3):
                        mm = ps.tile([128, 512], f32, tag="mm", bufs=2, name="mm")
                        for kc in range(KC):
                            nc.tensor.matmul(
                                mm[:, :nt],
                                wt_sb[:, mc, kc, :],
                                xt_tile[:, kc, :nt],
                                start=(kc == 0),
                                stop=(kc == KC - 1),
                            )
                        nc.vector.tensor_scalar_add(
                            qkvT[mc][:, t0 : t0 + nt],
                            mm[:, :nt],
                            bias_sb[:, mc : mc + 1],
                        )
                        if mc == 2:
                            state["ti_done"] = ti + 1
                        yield

            feeder = _qkv_groups()

            n_groups = 3 * n_tok_chunks

            n_iters = sum(
                ((L + 511) // 512) * ((L + 127) // 128 + 1) for _, L in units
            )
            pace = {"acc": 0.0, "groups": n_groups, "iters": n_iters}

            def feed(n):
                for _ in range(n):
                    if next(feeder, "done") == "done":
                        break
                    pace["groups"] -= 1

            # --- va refresh pipeline (one unit ahead of attention) ---
            prep_q = deque()  # (unit_idx, O, L, sid, jc)

            def enqueue_preps(ui):
                if ui >= len(units):
                    return
                O, L = units[ui]
                for jc in range((L + 127) // 128):
                    prep_q.append((ui, O, L, jc))

            def emit_prep(item):
                ui, O, L, jc = item
                nj = min(128, L - jc * 128)
                c0 = O + jc * 128
                # one full-height transpose yields BOTH heads' v block:
                # vps[k, 0:64] = head0 dims, vps[k, 64:128] = head1 dims
                vps = ps.tile([128, 128], bf16, tag="mm", bufs=2, name="vps")
                nc.tensor.transpose(
                    vps[:nj, :],
                    vT[:, c0 : c0 + nj],
                    ident_bf[:, :],
                )
                # one copy fans both halves out to the per-head v columns
                nc.vector.tensor_copy(
                    va[(ui % 2, jc)][:nj, :, 64:128], vps[:nj, :]
                )

            def prep_ready(item):
                ui, O, L, jc = item
                end_tok = O + jc * 128 + min(128, L - jc * 128)
                return state["ti_done"] * 512 >= end_tok

            def drain_preps(limit):
                n = 0
                while prep_q and n < limit and prep_ready(prep_q[0]):
                    emit_prep(prep_q.popleft())
                    n += 1

            def force_preps(ui):
                # everything queued for units <= ui must be emitted now
                while prep_q and prep_q[0][0] <= ui:
                    if not prep_ready(prep_q[0]):
                        feed(1)
                        continue
                    emit_prep(prep_q.popleft())

            def feed_cb():
                if pace["iters"] > 0:
                    # slight over-pace: drain the feeder a bit early so the
                    # late (tiny-unit) beats aren't competing with QKV
                    pace["acc"] += 1.25 * pace["groups"] / pace["iters"]
                pace["iters"] -= 1
                k = min(int(pace["acc"]), pace["groups"])
                if k > 0:
                    pace["acc"] -= k
                    feed(k)
                drain_preps(2)
                if pace["groups"] == 0 and not prep_q:
                    # feeder dry: a PE keepalive holds the HAM clock gate
                    # open through the ACT-bound tail (small so it doesn't
                    # delay the real beat work queued behind it)
                    dm = ps.tile([128, 512], f32, tag="mm", bufs=2, name="dm")
                    nc.tensor.matmul(
                        dm[:, 0:256],
                        wt_sb[:, 0, 0, :],
                        qT[:, 0:256],
                        start=True,
                        stop=True,
                    )

            # --- attention: units descending, both heads fused per beat ---
            enqueue_preps(0)
            for ui, (O, L) in enumerate(units):
                nk = (L + 127) // 128
                need = min(n_tok_chunks * 512, O + L + 512)
                while state["ti_done"] * 512 < need:
                    feed(1)
                force_preps(ui)
                enqueue_preps(ui + 1)
                par = ui % 2
                for q0 in range(0, L, 512):
                    nq = min(512, L - q0)
                    ovs = [
                        ps.tile([128, 512], f32, tag="ov", bufs=3, name="ov")
                        for _ in range(HEADS_PER_CORE)
                    ]
                    def emit_outs(pend):
                        pjc, pnj, pess = pend
                        for sid in range(HEADS_PER_CORE):
                            nc.tensor.matmul(
                                ovs[sid][:, :nq],
                                va[(par, pjc)][:pnj, sid, :],
                                pess[sid][:pnj, :nq],
                                start=(pjc == 0),
                                stop=(pjc == nk - 1),
                            )

                    # beats are software-pipelined two steps: the outs for
                    # beat jc-2 are emitted after beat jc's scores+exp, so
                    # they never sit in the in-order PE queue waiting on a
                    # recently-issued exp
                    pend = deque()
                    for jc in range(nk):
                        nj = min(128, L - jc * 128)
                        c0 = O + jc * 128
                        sps = [
                            ps.tile([128, 512], f32, tag="sc", bufs=3, name="sps")
                            for _ in range(HEADS_PER_CORE)
                        ]
                        ess = [
                            esp.tile([128, 512], bf16, tag="es", name="es")
                            for _ in range(HEADS_PER_CORE)
                        ]
                        for sid in range(HEADS_PER_CORE):
                            p0 = D * sid
                            nc.tensor.matmul(
                                sps[sid][:nj, :nq],
                                kT[p0 : p0 + D, c0 : c0 + nj],
                                qT[p0 : p0 + D, O + q0 : O + q0 + nq],
                                start=True,
                                stop=True,
                            )
                        for sid in range(HEADS_PER_CORE):
                            nc.scalar.activation(
                                ess[sid][:nj, :nq],
                                sps[sid][:nj, :nq],
                                Exp,
                                scale=0.125,
                            )
                        feed_cb()
                        if len(pend) >= 1:
                            emit_outs(pend.popleft())
                        pend.append((jc, nj, ess))
                    while pend:
                        emit_outs(pend.popleft())
                    feed_cb()
                    # normalize per head in the transposed layout:
                    # recip of den row (PSUM partition 0) on DVE, GpSimd
                    # partition_broadcast, DVE multiply (cast bf16), then a
                    # row-contiguous DMA (gpsimd queue -- keeps the big xt
                    # stream's queue free of output traffic).
                    ot = obp.tile([128, 512], bf16, tag="ob", name="ot")
                    for sid in range(HEADS_PER_CORE):
                        ov = ovs[sid]
                        rs = rsp.tile([1, 512], f32, tag="rs", name="rs")
                        nc.vector.reciprocal_approx_fast(
                            rs[:, :nq], ov[0:1, :nq]
                        )
                        rb = rbp.tile([64, 512], f32, tag="rb", name="rb")
                        nc.gpsimd.partition_broadcast(rb[:, :nq], rs[:, :nq])
                        nc.vector.tensor_tensor(
                            ot[D * sid : D * sid + D, :nq],
                            ov[64:128, :nq],
                            rb[:, :nq],
                            Mult,
                        )
                    # both heads' rows in one tile -> a single output DMA
                    nc.sync.dma_start(
                        out[:, O + q0 : O + q0 + nq], ot[:, :nq]
                    )

            feed(n_groups)  # drain any leftovers

    nc.compile()
    return nc


def _prepare(hidden_states, Wqkv_weight, Wqkv_bias, cu_seqlens):
    """Host-side sharding prep. Returns (nc, in_maps, assemble)."""
    hs = np.asarray(hidden_states, dtype=np.float32)
    W = np.asarray(Wqkv_weight, dtype=np.float32)
    b = np.asarray(Wqkv_bias, dtype=np.float32).reshape(-1)
    cs = np.asarray(cu_seqlens).astype(np.int64).reshape(-1)
    nnz, dim = hs.shape
    assert dim == DIM and W.shape == (3 * DIM, DIM)
    lengths = [int(cs[i + 1] - cs[i]) for i in range(len(cs) - 1)]
    assert sum(lengths) == nnz, (lengths, nnz)

    # permute sequences to descending length, except one medium sequence
    # moved to the very end so the drain phase has dense PE work instead
    # of a few trivial beats
    order = sorted(range(len(lengths)), key=lambda i: -lengths[i])
    if len(order) > 4:
        mid = len(order) // 2
        order = order[:mid] + order[mid + 1 :] + [order[mid]]
    tok_perm = np.concatenate(
        [np.arange(cs[i], cs[i] + lengths[i], dtype=np.int64) for i in order]
    ) if lengths else np.arange(0, dtype=np.int64)
    sorted_lengths = tuple(lengths[i] for i in order)
    hs_p = hs[tok_perm]

    nc = _build(nnz, sorted_lengths)

    # xt pre-tiled: [n_chunks, 128(partition), 8(kc), 512(token)] where
    # element [ti, p, a, j] = hs_p[512*ti + j, 128*a + p]
    NT = (nnz + 511) // 512
    hs_pad = hs_p
    if NT * 512 != nnz:
        hs_pad = np.concatenate(
            [hs_p, np.zeros((NT * 512 - nnz, DIM), dtype=np.float32)], axis=0
        )
    xt_np = np.ascontiguousarray(
        hs_pad.reshape(NT, 512, DIM // 128, 128).transpose(0, 3, 2, 1)
    ).astype(ml_dtypes.bfloat16)
    in_maps = []
    for c in range(N_CORES):
        r0 = c * HEADS_PER_CORE * HEAD_DIM  # 128c
        # wt mc-major: [3(mc), 128(partition=k within chunk), 8(kc), 128(m)]
        # element [mc, p, a, m] = W[mc*1024 + r0 + m, 128*a + p]
        wt_np = np.empty((3, 128, DIM // 128, 128), dtype=ml_dtypes.bfloat16)
        biases = []
        for mc in range(3):
            Wc = W[mc * DIM + r0 : mc * DIM + r0 + 128, :]  # (128m, 1024)
            wt_np[mc] = np.ascontiguousarray(
                Wc.T.reshape(DIM // 128, 128, 128).transpose(1, 0, 2)
            ).astype(ml_dtypes.bfloat16)
            biases.append(b[mc * DIM + r0 : mc * DIM + r0 + 128])
        bias_np = np.ascontiguousarray(np.stack(biases, axis=0).T)  # (128, 3)
        in_maps.append({"xt": xt_np, "wt": wt_np, "bias": bias_np})

    def assemble(results):
        full = np.empty((nnz, DIM), dtype=np.float32)
        for c in range(N_CORES):
            full[:, c * 128 : (c + 1) * 128] = (
                np.asarray(results[c]["out"]).astype(np.float32).T
            )
        outp = np.empty_like(full)
        outp[tok_perm] = full
        return outp

    return nc, in_maps, assemble


def kernel(hidden_states, Wqkv_weight, Wqkv_bias, cu_seqlens, max_seqlen=None):
    from concourse.bass_utils import run_bass_kernel_spmd

    nc, in_maps, assemble = _prepare(
        hidden_states, Wqkv_weight, Wqkv_bias, cu_seqlens
    )
    res = run_bass_kernel_spmd(nc, in_maps, list(range(N_CORES)))
    return assemble(res.results)


# revision 57
# speedup vs baseline: 1.0133x; 1.0133x over previous
"""Trainium2 Bass kernel for varlen (ragged) BERT self-attention.

Strategy: tensor-parallel over heads. 16 heads across 8 NeuronCores ->
2 heads per core. Every core runs an IDENTICAL program (SPMD) on:
  - xt:   full hidden_states (sequences host-permuted to DESCENDING
          length), pre-transposed+cast to bf16, tiled [NT, 128, 8, 512]
  - wt:   this core's slice of Wqkv as matmul-lhsT blocks, mc-major
          [3, 128, 8, 128] bf16 (per-stripe DMA -> first matmul sooner)
  - bias: this core's bias slice (128, 3) f32
Output per core: (128, nnz) bf16 = the 2 owned heads' output rows,
TRANSPOSED, in permuted token order. Host transposes back, concatenates,
and un-permutes.

On-chip per core:
  1. QKV projection: Y^T[384, nnz] = Wc @ X^T, K=1024 in 8 chunks,
     bias added during PSUM->SBUF eviction (DVE tensor_scalar add),
     cast bf16 -> qT/kT/vT resident in SBUF as [128(=2hx64), nnz].
  2. Attention per sequence (descending length; NO packing, NO masks:
     only diagonal blocks are computed), one "beat" per (q-chunk of
     512, k-chunk jc of 128): the TWO heads' score matmuls (K=64) are
     issued back-to-back into opposite PE row groups (partitions 0-63
     vs 64-127 via auto tile_position) so they execute CONCURRENTLY;
     one shared exp on ACT over [nj, 2, nq] (scale 1/8 folded in);
     two out matmuls (va = [ones | pad | v] so the softmax denominator
     lands on PSUM partition 0 for free). Normalize per (q-chunk,
     head): DVE reciprocal of the denominator row, GpSimd
     partition_broadcast, DVE multiply (cast bf16), row-contiguous DMA.
  3. v_aug refresh (PE transpose of vT block + DVE copy) is PIPELINED
     INTO THE FEEDER STREAM one unit ahead of attention (va slots are
     parity double-buffered per unit), so unit boundaries never starve
     the PE -- PE starvation is what used to trip the HAM clock gate
     (k=4/8 half-clock windows). Keepalive matmuls still hold the gate
     open once the feeder runs dry in the tail.

Scheduling: the QKV chunk stream (PE-only work) is paced into the
beat stream (groups-remaining / beats-remaining), with sequences
DESCENDING so the deepest attention overlaps the longest feeder
stretch and the tail units are trivial.
"""

import functools
import sys

import numpy as np

for _p in ("/opt/trn_rl_repo",):
    if _p not in sys.path:
        sys.path.append(_p)

import ml_dtypes  # noqa: E402

N_HEADS = 16
HEAD_DIM = 64
DIM = 1024
N_CORES = 8
HEADS_PER_CORE = N_HEADS // N_CORES  # 2


@functools.lru_cache(maxsize=4)
def _build(nnz, lengths):
    """Build + compile the SPMD Bass program.

    lengths: per-sequence token counts, already in DESCENDING order
    (the host permutes sequences before tiling xt).
    """
    from collections import deque

    import concourse.mybir as mybir
    import concourse.tile as tile
    from concourse import bacc
    from concourse.masks import make_identity

    f32 = mybir.dt.float32
    bf16 = mybir.dt.bfloat16
    Exp = mybir.ActivationFunctionType.Exp
    Mult = mybir.AluOpType.mult

    KC = DIM // 128  # 8 contraction chunks
    D = HEAD_DIM
    n_tok_chunks = (nnz + 511) // 512

    units = []
    off = 0
    for L in lengths:
        if L:
            units.append((off, L))
            off += L
    assert off == nnz

    nc = bacc.Bacc("TRN2", target_bir_lowering=False, debug=False)
    xt = nc.declare_dram_parameter(
        "xt", [n_tok_chunks, 128, KC, 512], bf16, isOutput=False
    )
    wt = nc.declare_dram_parameter("wt", [3, 128, KC, 128], bf16, isOutput=False)
    bias = nc.declare_dram_parameter("bias", [128, 3], f32, isOutput=False)
    out = nc.declare_dram_parameter("out", [128, nnz], bf16, isOutput=True)

    with tile.TileContext(nc) as tc:
        with (
            tc.tile_pool(name="res", bufs=1) as res,
            tc.tile_pool(name="xp", bufs=8) as xp,
            tc.tile_pool(name="esp", bufs=6) as esp,
            tc.tile_pool(name="rsp", bufs=3) as rsp,
            tc.tile_pool(name="rbp", bufs=3) as rbp,
            tc.tile_pool(name="obp", bufs=6) as obp,
            tc.tile_pool(name="ps", bufs=1, space="PSUM") as ps,
        ):
            # startup DMAs interleaved by first-use order: the sync queue
            # serves wt stripe 0, then chunk0's second half, then stripe 1,
            # chunk1's second half, stripe 2, bias; the scalar queue serves
            # the chunks' first halves in parallel.
            wt_sb = res.tile([128, 3, KC, 128], bf16)
            bias_sb = res.tile([128, 3], f32)
            prefetched = {}
            for ti in (0, 1):
                prefetched[ti] = xp.tile([128, KC, 512], bf16, tag="xt", name="xt_t")
                nc.scalar.dma_start(prefetched[ti][:, 0:4, :], xt[ti, :, 0:4, :])
            nc.sync.dma_start(wt_sb[:, 0, :, :], wt[0, :, :, :])
            nc.sync.dma_start(prefetched[0][:, 4:8, :], xt[0, :, 4:8, :])
            nc.sync.dma_start(bias_sb[:], bias[:, :])
            nc.sync.dma_start(wt_sb[:, 1, :, :], wt[1, :, :, :])
            nc.sync.dma_start(wt_sb[:, 2, :, :], wt[2, :, :, :])
            nc.sync.dma_start(prefetched[1][:, 4:8, :], xt[1, :, 4:8, :])

            ident_bf = res.tile([128, 128], bf16)
            make_identity(nc, ident_bf[:])

            # PE warmup: dependency-free matmuls at t0 keep the tensor
            # engine busy (and its clock ramping) while the first DMAs are
            # still in flight
            warm = res.tile([128, 512], bf16)
            nc.vector.memset(warm[:, :], 0.0)
            for _ in range(20):
                wp = ps.tile([128, 512], f32, tag="mm", bufs=2, name="wp")
                nc.tensor.matmul(
                    wp[:, :], warm[:, 0:128], warm[:, :], start=True, stop=True
                )

            qT = res.tile([128, nnz], bf16)
            kT = res.tile([128, nnz], bf16)
            vT = res.tile([128, nnz], bf16)
            qkvT = (qT, kT, vT)

            # va slots, parity double-buffered per unit so the refresh for
            # unit u+1 can run during unit u with no WAR stall:
            # [ktok(128), ones(1)+pad(63)+v(64)]. Ones col 0 => softmax
            # denominator lands on PSUM partition 0; v at cols 64..127 so
            # the normalize multiply reads PSUM partitions 64..127.
            max_nk = max((L + 127) // 128 for _, L in units)
            va = {}
            for par in range(2):
                for jc in range(max_nk):
                    # both heads share one tile: [ktok, head, ones|pad|v]
                    t = res.tile([128, 2, 128], bf16, name=f"va{par}{jc}")
                    nc.vector.memset(t[:, :, :], 0.0)
                    nc.vector.memset(t[:, :, 0:1], 1.0)
                    va[(par, jc)] = t

            # --- QKV feeder: one (ti, mc) matmul group per step ---
            state = {"ti_done": 0}

            def _qkv_groups():
                for ti in range(n_tok_chunks):
                    t0 = ti * 512
                    nt = min(512, nnz - t0)
                    if ti in prefetched:
                        xt_tile = prefetched[ti]
                    else:
                        xt_tile = xp.tile(
                            [128, KC, 512], bf16, tag="xt", name="xt_t"
                        )
                        nc.sync.dma_start(xt_tile[:, :, :], xt[ti, :, :, :])
                    for mc in range(3):
                        mm = ps.tile([128, 512], f32, tag="mm", bufs=2, name="mm")
                        for kc in range(KC):
                            nc.tensor.matmul(
                                mm[:, :nt],
                                wt_sb[:, mc, kc, :],
                                xt_tile[:, kc, :nt],
                                start=(kc == 0),
                                stop=(kc == KC - 1),
                            )
                        nc.vector.tensor_scalar_add(
                            qkvT[mc][:, t0 : t0 + nt],
                            mm[:, :nt],
                            bias_sb[:, mc : mc + 1],
                        )
                        if mc == 2:
                            state["ti_done"] = ti + 1
                        yield

            feeder = _qkv_groups()

            n_groups = 3 * n_tok_chunks

            n_iters = sum(
                ((L + 511) // 512) * ((L + 127) // 128 + 1) for _, L in units
            )
            pace = {"acc": 0.0, "groups": n_groups, "iters": n_iters}

            def feed(n):
                for _ in range(n):
                    if next(feeder, "done") == "done":
                        break
                    pace["groups"] -= 1

            # --- va refresh pipeline (one unit ahead of attention) ---
            prep_q = deque()  # (unit_idx, O, L, sid, jc)

            def enqueue_preps(ui):
                if ui >= len(units):
                    return
                O, L = units[ui]
                for jc in range((L + 127) // 128):
                    prep_q.append((ui, O, L, jc))

            def emit_prep(item):
                ui, O, L, jc = item
                nj = min(128, L - jc * 128)
                c0 = O + jc * 128
                # one full-height transpose yields BOTH heads' v block:
                # vps[k, 0:64] = head0 dims, vps[k, 64:128] = head1 dims
                vps = ps.tile([128, 128], bf16, tag="mm", bufs=2, name="vps")
                nc.tensor.transpose(
                    vps[:nj, :],
                    vT[:, c0 : c0 + nj],
                    ident_bf[:, :],
                )
                # one copy fans both halves out to the per-head v columns
                nc.vector.tensor_copy(
                    va[(ui % 2, jc)][:nj, :, 64:128], vps[:nj, :]
                )

            def prep_ready(item):
                ui, O, L, jc = item
                end_tok = O + jc * 128 + min(128, L - jc * 128)
                return state["ti_done"] * 512 >= end_tok

            def drain_preps(limit):
                n = 0
                while prep_q and n < limit and prep_ready(prep_q[0]):
                    emit_prep(prep_q.popleft())
                    n += 1

            def force_preps(ui):
                # everything queued for units <= ui must be emitted now
                while prep_q and prep_q[0][0] <= ui:
                    if not prep_ready(prep_q[0]):
                        feed(1)
                        continue
                    emit_prep(prep_q.popleft())

            def feed_cb():
                if pace["iters"] > 0:
                    # slight over-pace: drain the feeder a bit early so the
                    # late (tiny-unit) beats aren't competing with QKV
                    pace["acc"] += 1.25 * pace["groups"] / pace["iters"]
                pace["iters"] -= 1
                k = min(int(pace["acc"]), pace["groups"])
                if k > 0:
                    pace["acc"] -= k
                    feed(k)
                drain_preps(2)
                if pace["groups"] == 0 and not prep_q:
                    # feeder dry: a PE keepalive holds the HAM clock gate
                    # open through the ACT-bound tail (small so it doesn't
                    # delay the real beat work queued behind it)
                    dm = ps.tile([128, 512], f32, tag="mm", bufs=2, name="dm")
                    nc.tensor.matmul(
                        dm[:, 0:256],
                        wt_sb[:, 0, 0, :],
                        qT[:, 0:256],
                        start=True,
                        stop=True,
                    )

            # --- attention: units descending, both heads fused per beat ---
            enqueue_preps(0)
            for ui, (O, L) in enumerate(units):
                nk = (L + 127) // 128
                need = min(n_tok_chunks * 512, O + L + 512)
                while state["ti_done"] * 512 < need:
                    feed(1)
                force_preps(ui)
                enqueue_preps(ui + 1)
                par = ui % 2
                for q0 in range(0, L, 512):
                    nq = min(512, L - q0)
                    ovs = [
                        ps.tile([128, 512], f32, tag="ov", bufs=3, name="ov")
                        for _ in range(HEADS_PER_CORE)
                    ]
                    def emit_outs(pend):
                        pjc, pnj, pess = pend
                        for sid in range(HEADS_PER_CORE):
                            nc.tensor.matmul(
                                ovs[sid][:, :nq],
                                va[(par, pjc)][:pnj, sid, :],
                                pess[sid][:pnj, :nq],
                                start=(pjc == 0),
                                stop=(pjc == nk - 1),
                            )

                    # beats are software-pipelined two steps: the outs for
                    # beat jc-2 are emitted after beat jc's scores+exp, so
                    # they never sit in the in-order PE queue waiting on a
                    # recently-issued exp
                    pend = deque()
                    for jc in range(nk):
                        nj = min(128, L - jc * 128)
                        c0 = O + jc * 128
                        sps = [
                            ps.tile([128, 512], f32, tag="sc", bufs=3, name="sps")
                            for _ in range(HEADS_PER_CORE)
                        ]
                        ess = [
                            esp.tile([128, 512], bf16, tag="es", name="es")
                            for _ in range(HEADS_PER_CORE)
                        ]
                        for sid in range(HEADS_PER_CORE):
                            p0 = D * sid
                            nc.tensor.matmul(
                                sps[sid][:nj, :nq],
                                kT[p0 : p0 + D, c0 : c0 + nj],
                                qT[p0 : p0 + D, O + q0 : O + q0 + nq],
                                start=True,
                                stop=True,
                            )
                        for sid in range(HEADS_PER_CORE):
                            nc.scalar.activation(
                                ess[sid][:nj, :nq],
                                sps[sid][:nj, :nq],
                                Exp,
                                scale=0.125,
                            )
                        feed_cb()
                        if len(pend) >= 1:
                            emit_outs(pend.popleft())
                        pend.append((jc, nj, ess))
                    while pend:
                        emit_outs(pend.popleft())
                    feed_cb()
                    # normalize per head in the transposed layout:
                    # recip of den row (PSUM partition 0) on DVE, GpSimd
                    # partition_broadcast, DVE multiply (cast bf16), then a
                    # row-contiguous DMA (gpsimd queue -- keeps the big xt
                    # stream's queue free of output traffic).
                    ot = obp.tile([128, 512], bf16, tag="ob", name="ot")
                    for sid in range(HEADS_PER_CORE):
                        ov = ovs[sid]
                        rs = rsp.tile([1, 512], f32, tag="rs", name="rs")
                        nc.vector.reciprocal_approx_fast(
                            rs[:, :nq], ov[0:1, :nq]
                        )
                        rb = rbp.tile([64, 512], f32, tag="rb", name="rb")
                        nc.gpsimd.partition_broadcast(rb[:, :nq], rs[:, :nq])
                        nc.vector.tensor_tensor(
                            ot[D * sid : D * sid + D, :nq],
                            ov[64:128, :nq],
                            rb[:, :nq],
                            Mult,
                        )
                    # both heads' rows in one tile -> a single output DMA
                    nc.sync.dma_start(
                        out[:, O + q0 : O + q0 + nq], ot[:, :nq]
                    )

            feed(n_groups)  # drain any leftovers

    nc.compile()
    return nc


def _prepare(hidden_states, Wqkv_weight, Wqkv_bias, cu_seqlens):
    """Host-side sharding prep. Returns (nc, in_maps, assemble)."""
    hs = np.asarray(hidden_states, dtype=np.float32)
    W = np.asarray(Wqkv_weight, dtype=np.float32)
    b = np.asarray(Wqkv_bias, dtype=np.float32).reshape(-1)
    cs = np.asarray(cu_seqlens).astype(np.int64).reshape(-1)
    nnz, dim = hs.shape
    assert dim == DIM and W.shape == (3 * DIM, DIM)
    lengths = [int(cs[i + 1] - cs[i]) for i in range(len(cs) - 1)]
    assert sum(lengths) == nnz, (lengths, nnz)

    # permute sequences to descending length, except one medium sequence
    # moved to the very end so the drain phase has dense PE work instead
    # of a few trivial beats
    order = sorted(range(len(lengths)), key=lambda i: -lengths[i])
    if len(order) > 4:
        mid = len(order) // 2
        order = order[:mid] + order[mid + 1 :] + [order[mid]]
    tok_perm = np.concatenate(
        [np.arange(cs[i], cs[i] + lengths[i], dtype=np.int64) for i in order]
    ) if lengths else np.arange(0, dtype=np.int64)
    sorted_lengths = tuple(lengths[i] for i in order)
    hs_p = hs[tok_perm]

    nc = _build(nnz, sorted_lengths)

    # xt pre-tiled: [n_chunks, 128(partition), 8(kc), 512(token)] where
    # element [ti, p, a, j] = hs_p[512*ti + j, 128*a + p]
    NT = (nnz + 511) // 512
    hs_pad = hs_p
    if NT * 512 != nnz:
        hs_pad = np.concatenate(
            [hs_p, np.zeros((NT * 512 - nnz, DIM), dtype=np.float32)], axis=0
        )
    xt_np = np.ascontiguousarray(
        hs_pad.reshape(NT, 512, DIM // 128, 128).transpose(0, 3, 2, 1)
    ).astype(ml_dtypes.bfloat16)
    in_maps = []
    for c in range(N_CORES):
        r0 = c * HEADS_PER_CORE * HEAD_DIM  # 128c
        # wt mc-major: [3(mc), 128(partition=k within chunk), 8(kc), 128(m)]
        # element [mc, p, a, m] = W[mc*1024 + r0 + m, 128*a + p]
        wt_np = np.empty((3, 128, DIM // 128, 128), dtype=ml_dtypes.bfloat16)
        biases = []
        for mc in range(3):
            Wc = W[mc * DIM + r0 : mc * DIM + r0 + 128, :]  # (128m, 1024)
            wt_np[mc] = np.ascontiguousarray(
                Wc.T.reshape(DIM // 128, 128, 128).transpose(1, 0, 2)
            ).astype(ml_dtypes.bfloat16)
            biases.append(b[mc * DIM + r0 : mc * DIM + r0 + 128])
        bias_np = np.ascontiguousarray(np.stack(biases, axis=0).T)  # (128, 3)
        in_maps.append({"xt": xt_np, "wt": wt_np, "bias": bias_np})

    def assemble(results):
        full = np.empty((nnz, DIM), dtype=np.float32)
        for c in range(N_CORES):
            full[:, c * 128 : (c + 1) * 128] = (
                np.asarray(results[c]["out"]).astype(np.float32).T
            )
        outp = np.empty_like(full)
        outp[tok_perm] = full
        return outp

    return nc, in_maps, assemble


def kernel(hidden_states, Wqkv_weight, Wqkv_bias, cu_seqlens, max_seqlen=None):
    from concourse.bass_utils import run_bass_kernel_spmd

    nc, in_maps, assemble = _prepare(
        hidden_states, Wqkv_weight, Wqkv_bias, cu_seqlens
    )
    res = run_bass_kernel_spmd(nc, in_maps, list(range(N_CORES)))
    return assemble(res.results)
